# revision 11
# baseline (speedup 1.0000x reference)
"""Trainium2 Bass kernel for nn_DifferentiablePathfinder.

Reference computation (N=8192, 20 iterations, tau=0.1):
    d0 = where(mask>0, 0, 100)
    effw = where(adj>0, W, 100)
    repeat 20x: d = min(d, -tau * logsumexp(-(d[:,None] + effw)/tau, axis=0))

Reformulation in linear ("q") space: with E = exp(-effw/tau) (zero where no
edge) and q = exp(-d/tau), one iteration is exactly

    q <- max(q, E^T q)        (elementwise max == min in d-space)

i.e. a repeated matvec with a FIXED matrix.  d never converges here (softmin
over ~4k candidates drags every distance down ~0.6/iter), so q would overflow
f32.  We rescale q every iteration (alternating 2^-9 / 2^-8, exact in fp,
which also keeps q in fp8's normal range [~0.7, ~7]) and track the
accumulated offset as a compile-time constant:

    stored q_t = exp(-(d_t - m_t)/tau),  m_{t+1} = m_t + tau*ln(scale_t)
    q_{t+1} = max(q_t, E^T q_t) * scale_t
    final d = m_T - tau * ln(q_T)

Sharding: E is column-sharded across 8 cores (1024 cols each).  The host
pre-merges adjacency+weights into ew = where(adj>0, W, 100) in bf16 (pure
input prep; 16 MB/core instead of 64 MB of f32 W + int32 adj, cutting the
initial HBM load 4x).  Each core keeps its [8192, 1024] block of
E = exp(-ew/tau) resident in SBUF as fp8-e4m3 (8 MB, built by the scalar
engine's Exp directly into the fp8 DoubleRow plane layout), and computes
s = E^T q on the tensor engine in DoubleRow mode (32 K-chunks of 256 rows
accumulated in PSUM f32).

Wave-pipelined AllGather: the per-iteration AG roundtrip (~8-10 us: DVE
scale+cast, DMA to the DRAM bounce buffer, TOPSP trigger, ~5 us collective,
DMA back, max) is hidden behind the ~17 us of matmuls by splitting each
iteration into two column waves:

  - q[p*64+k] lives at SBUF partition p, col k, split as q8a (k<32) and
    q8b (k>=32).  DoubleRow chunk c2 pairs cols (c2, c2+16) for c2<16 and
    (c2+16, c2+32) for c2>=16 (16 B apart - the minimum aligned interleave
    step), so chunks 0..15 read only q8a and 16..31 only q8b.
  - output columns are stored/computed u-ordered: group A = {j: j%64<32}
    (these feed every core's q8a), group B = the rest.  After AllGather of
    a group, each SBUF row's 32 bytes are contiguous in the gather buffer
    (flat addr = 32p + k), so the receive is a single clean DMA.
  - schedule per iteration: psA chunks 0..15 | psB 0..7 | psA 16..31 ->
    AG_A fires at ~60% of the burst | psB 8..31 -> AG_B at the end.  The
    next iteration needs AG_A at its start and AG_B only ~6 us in.
  - the elementwise max runs on the RECEIVE side ([128,32] tiles, 128-way
    parallel) against the pre-scaled previous q; receive DMA + max sit on
    the same (gpsimd) queue to avoid a cross-engine semaphore hop.

The first collective also absorbs the one-time ~55-80 us cross-core
dispatch-skew barrier; the E build + iteration 0 (~60 us) overlap most of
it, and no other work depends on a collective before that point.

Accuracy vs f32 reference: ~3e-4 relative (fp8 E quantization dominates;
errors average over ~2k terms per dot product; bf16 ew adds ~0.2% weight
rounding, far below fp8's 3%).

NOTE: all DRAM tensors and every AP passed to DMA are kept strictly 2-D+ -
1-D APs (e.g. `t[0, :]`) produce NEFFs that fail to load / wedge the device
on this environment.  tensor_tensor_reduce also fails at runtime here; use
separate max + scale ops.
"""

import numpy as np

# ---------------------------------------------------------------- constants
N = 8192
CORES = 8
COLS = N // CORES          # 1024 columns per core
P = 128                    # partitions
KPP = N // P               # 64 q entries per partition
CH2 = KPP // 2             # 32 DoubleRow chunks of 256 rows
HALF = COLS // 2           # 512 (output-group size / PSUM bank)
T = 20                     # iterations (fixed; reference never converges)
TAU = 0.1
INF_W = 100.0              # no-edge marker in ew (exp(-1000) == 0 in fp8)
SCALES = [1.0 / 512.0 if t % 2 == 0 else 1.0 / 256.0 for t in range(T)]
M_T = TAU * float(np.sum(np.log(SCALES)))   # log-offset after T iters

RPS = 4                    # rows per slab (per partition)
NSLAB = KPP // RPS         # 16 slabs


def _slab_dest(s):
    """E3 (chunk-range start, plane) written by slab s (rows k = 4s..4s+3)."""
    k0 = s * RPS
    if k0 < 16:
        return k0, 0            # c2 = k, plane 0
    if k0 < 32:
        return k0 - 16, 1       # c2 = k-16, plane 1
    if k0 < 48:
        return k0 - 16, 0       # c2 = k-16, plane 0
    return k0 - 32, 1           # c2 = k-32, plane 1


_CACHE = {}


def _build():
    """Build + compile the SPMD Bass program (same program on all 8 cores)."""
    import concourse.bacc as bacc
    import concourse.mybir as mybir
    import concourse.tile as tile

    f32 = mybir.dt.float32
    bf16 = mybir.dt.bfloat16
    fp8 = mybir.dt.float8e4
    i32 = mybir.dt.int32
    DR = mybir.MatmulPerfMode.DoubleRow

    nc = bacc.Bacc(
        "TRN2",
        target_bir_lowering=False,
        debug=False,
        enable_asserts=False,
        num_devices=CORES,
    )

    ew_dram = nc.dram_tensor("ew_block", [N, COLS], bf16, kind="ExternalInput")
    maskown_dram = nc.dram_tensor("mask_own", [1, COLS], i32, kind="ExternalInput")
    maskfull_dram = nc.dram_tensor("mask_full", [1, N], i32, kind="ExternalInput")
    d_dram = nc.dram_tensor("d_out", [1, COLS], f32, kind="ExternalOutput")

    # slab view: slab s holds rows {p*64 + 4s + r : r in 0..3} on partition p -
    # 4 consecutive rows per partition = one contiguous 8 KB DRAM run per
    # partition (bigger runs lift the DMA-engine rate substantially)
    ew_r = ew_dram.rearrange("(p s r) c -> s p (r c)", s=NSLAB, r=RPS)

    with tile.TileContext(nc) as tc:
        with (
            tc.tile_pool(name="resident", bufs=1) as rpool,
            tc.tile_pool(name="stage", bufs=3) as spool,
            tc.tile_pool(name="qpool", bufs=2) as qpool,
            tc.tile_pool(name="psum", bufs=2, space="PSUM") as ppool,
            tc.tile_pool(name="dram", bufs=2, space="DRAM") as dpool,
        ):
            # resident E block, fp8 DoubleRow planes: 64 KB/partition.
            # columns u-ordered: u<512 <-> output group A (j = 64*(u//32)+u%32)
            E3 = rpool.tile([P, CH2, 2, COLS], fp8)

            # ---------------- initial q from source mask (no collective) --
            maskown_sb = spool.tile([1, COLS], i32, tag="mskown", bufs=1)
            nc.sync.dma_start(maskown_sb[0:1, :], maskown_dram[0:1, :])
            qp = qpool.tile([1, COLS], f32, tag="qp")
            nc.vector.tensor_copy(qp[0:1, :], maskown_sb[0:1, :])  # int32 -> f32

            mskfull_sb = spool.tile([P, KPP], i32, tag="mskfull", bufs=1)
            nc.sync.dma_start(
                mskfull_sb[:, :],
                maskfull_dram.rearrange("a (p k) -> (a p) k", k=KPP),
            )
            q8a = qpool.tile([P, CH2], fp8, tag="q8a")
            q8b = qpool.tile([P, CH2], fp8, tag="q8b")
            nc.vector.tensor_copy(q8a[:, :], mskfull_sb[:, 0:CH2])   # i32 -> fp8
            nc.vector.tensor_copy(q8b[:, :], mskfull_sb[:, CH2:KPP])

            # ---------------- build resident E = exp(-ew/tau) -------------
            # slab order pairs plane-0/plane-1 sources so DoubleRow chunks
            # become ready in schedule order; iteration 0 overlaps the build
            slab_order = []
            for s in range(4):
                slab_order += [s, s + 4]
            for s in range(8, 12):
                slab_order += [s, s + 4]
            slab_tiles = {}
            for i, s in enumerate(slab_order):
                ewst = spool.tile([P, RPS * COLS], bf16, tag="ewst", bufs=7)
                # alternate HW-DGE (sync) and SW-DGE (gpsimd) queues
                if i % 2 == 0:
                    nc.sync.dma_start(ewst[:, :], ew_r[s])
                else:
                    nc.gpsimd.dma_start(ewst[:, :], ew_r[s])
                slab_tiles[i] = (s, ewst)

            def emit_act(i, g):
                s, ewst = slab_tiles[i]
                c0, pl = _slab_dest(s)
                ewst4 = ewst.rearrange("p (r b j) -> p r b j", r=RPS, j=KPP)
                nc.scalar.activation(
                    E3[:, c0:c0 + RPS, pl, g * HALF:(g + 1) * HALF]
                    .rearrange("p c (b j) -> p c b j", j=CH2),
                    ewst4[:, :, :, g * CH2:(g + 1) * CH2],
                    mybir.ActivationFunctionType.Exp,
                    bias=0.0, scale=-1.0 / TAU,
                )

            # A-wave (output group 0) exps run ~5 slabs ahead of B-wave, so
            # iteration 0's psA - and with it the first real AllGather -
            # completes ~10 us earlier.  7 staging bufs bound the lookahead.
            LOOK = 5
            for i in range(NSLAB):
                emit_act(i, 0)
                if i >= LOOK:
                    emit_act(i - LOOK, 1)
            for i in range(NSLAB - LOOK, NSLAB):
                emit_act(i, 1)

            # ---------------- 20 iterations ------------------------------
            # chunk c2 -> q8a cols (c2, c2+16) for c2<16; q8b (c2-16, c2)
            def lhsT_of(c2):
                if c2 < 16:
                    return q8a[:, c2:c2 + 17:16].rearrange(
                        "p (a m) -> p a m", a=2)
                b0 = c2 - 16
                return q8b[:, b0:b0 + 17:16].rearrange("p (a m) -> p a m", a=2)

            def mm_group(ps, grp, c2s):
                for c2 in c2s:
                    nc.tensor.matmul(
                        ps[0:1, :], lhsT_of(c2),
                        E3[:, c2, :, grp * HALF:(grp + 1) * HALF],
                        start=(c2 == 0), stop=(c2 == CH2 - 1),
                        perf_mode=DR,
                    )

            for t in range(T):
                ps_a = ppool.tile([1, HALF], f32, tag="psa")
                ps_b = ppool.tile([1, HALF], f32, tag="psb")
                last = t == T - 1

                # pre-scaled previous q for the receive-side max; DVE runs
                # these while the matmuls stream
                if not last:
                    q8sa = qpool.tile([P, CH2], fp8, tag="q8sa")
                    q8sb = qpool.tile([P, CH2], fp8, tag="q8sb")
                    nc.vector.tensor_scalar_mul(q8sa[:, :], q8a[:, :], SCALES[t])
                    nc.vector.tensor_scalar_mul(q8sb[:, :], q8b[:, :], SCALES[t])

                # ---- matmul schedule: A-wave output first, B-chunks late
                mm_group(ps_a, 0, range(0, 16))
                mm_group(ps_b, 1, range(0, 8))
                mm_group(ps_a, 0, range(16, 32))
                # tail A: one scale+cast, DMA out, trigger (sync queue)
                if not last:
                    q8cca = qpool.tile([1, HALF], fp8, tag="q8cca")
                    nc.vector.tensor_scalar_mul(q8cca[0:1, :], ps_a[0:1, :], SCALES[t])
                    cc_ina = dpool.tile([1, HALF], fp8, tag="ccina")
                    nc.sync.dma_start(cc_ina[0:1, :], q8cca[0:1, :])
                    cc_outa = dpool.tile([CORES, HALF], fp8, tag="ccouta",
                                         addr_space="Shared")
                    nc.gpsimd.collective_compute(
                        "AllGather", mybir.AluOpType.bypass,
                        replica_groups=[list(range(CORES))],
                        ins=[cc_ina[0:1, :].opt()],
                        outs=[cc_outa[:, :].opt()],
                    )
                mm_group(ps_b, 1, range(8, 32))
                # HAM warm-keepers: the PE idles ~5 us waiting for AG_A's
                # roundtrip; >3.4 us idle drops the clock gate to 4/8 and the
                # next burst pays ~10 cold matmuls.  A dozen throwaway
                # matmuls (no deps, results unread) bridge the activity
                # window without flooding the 64-deep queue.
                if not last:
                    ps_d = ppool.tile([1, HALF], f32, tag="psd", bufs=1)
                    for _ in range(12):
                        nc.tensor.matmul(
                            ps_d[0:1, :], lhsT_of(0),
                            E3[:, 0, :, 0:HALF],
                            start=True, stop=True, perf_mode=DR,
                        )
                if not last:
                    q8ccb = qpool.tile([1, HALF], fp8, tag="q8ccb")
                    nc.vector.tensor_scalar_mul(q8ccb[0:1, :], ps_b[0:1, :], SCALES[t])
                    cc_inb = dpool.tile([1, HALF], fp8, tag="ccinb")
                    nc.sync.dma_start(cc_inb[0:1, :], q8ccb[0:1, :])
                    cc_outb = dpool.tile([CORES, HALF], fp8, tag="ccoutb",
                                         addr_space="Shared")
                    nc.gpsimd.collective_compute(
                        "AllGather", mybir.AluOpType.bypass,
                        replica_groups=[list(range(CORES))],
                        ins=[cc_inb[0:1, :].opt()],
                        outs=[cc_outb[:, :].opt()],
                    )
                    # receive: DMA on the (idle) scalar queue so neither
                    # the gpsimd trigger queue nor the DVE stalls; wave A
                    # first - it gates the next burst
                    agta = qpool.tile([P, CH2], fp8, tag="agta")
                    nc.scalar.dma_start(
                        agta[:, :],
                        cc_outa.rearrange("c (pp k) -> (c pp) k", k=CH2),
                    )
                    q8a_new = qpool.tile([P, CH2], fp8, tag="q8a")
                    nc.vector.tensor_tensor(
                        q8a_new[:, :], agta[:, :], q8sa[:, :],
                        mybir.AluOpType.max)
                    agtb = qpool.tile([P, CH2], fp8, tag="agtb")
                    nc.scalar.dma_start(
                        agtb[:, :],
                        cc_outb.rearrange("c (pp k) -> (c pp) k", k=CH2),
                    )
                    q8b_new = qpool.tile([P, CH2], fp8, tag="q8b")
                    nc.vector.tensor_tensor(
                        q8b_new[:, :], agtb[:, :], q8sb[:, :],
                        mybir.AluOpType.max)

                # ---- f32 master copy of own slice (j-ordered; feeds only
                # the final output, fully off the AG critical path)
                qp_s = qpool.tile([1, COLS], f32, tag="qps")
                nc.vector.tensor_scalar_mul(qp_s[0:1, :], qp[0:1, :], SCALES[t])
                qp_new = qpool.tile([1, COLS], f32, tag="qp")
                for grp, ps in ((0, ps_a), (1, ps_b)):
                    sps = qpool.tile([1, HALF], f32, tag="sps")
                    nc.vector.tensor_scalar_mul(sps[0:1, :], ps[0:1, :], SCALES[t])
                    jview = lambda ap: ap.rearrange(
                        "a (b j) -> a b j", j=KPP)[:, :, grp * CH2:(grp + 1) * CH2]
                    nc.vector.tensor_tensor(
                        jview(qp_new[0:1, :]), jview(qp_s[0:1, :]),
                        sps[0:1, :].rearrange("a (b j) -> a b j", j=CH2),
                        mybir.AluOpType.max,
                    )
                qp = qp_new
                if not last:
                    q8a = q8a_new
                    q8b = q8b_new

            # ---------------- final: d = m_T - tau*ln(q), clamp to 100 ----
            lnq = qpool.tile([1, COLS], f32, tag="lnq", bufs=1)
            nc.scalar.activation(
                lnq[0:1, :], qp[0:1, :], mybir.ActivationFunctionType.Ln,
            )
            dfin = qpool.tile([1, COLS], f32, tag="dfin", bufs=1)
            nc.scalar.activation(
                dfin[0:1, :], lnq[0:1, :], mybir.ActivationFunctionType.Copy,
                bias=M_T, scale=-TAU,
            )
            dcl = qpool.tile([1, COLS], f32, tag="dcl", bufs=1)
            nc.vector.tensor_scalar_min(dcl[0:1, :], dfin[0:1, :], 100.0)
            nc.sync.dma_start(d_dram[0:1, :], dcl[0:1, :])

    nc.compile()
    return nc


def _get_nc():
    if "nc" not in _CACHE:
        _CACHE["nc"] = _build()
    return _CACHE["nc"]


def _make_in_maps(adjacency, edge_weights, source_mask):
    import ml_dtypes

    adjacency = np.asarray(adjacency, dtype=np.int32)
    edge_weights = np.asarray(edge_weights, dtype=np.float32)
    source_mask = np.asarray(source_mask, dtype=np.int32)
    # input prep (pure sharding/packing): effective weights in bf16
    ew = np.where(adjacency > 0, edge_weights, np.float32(INF_W))
    ew = ew.astype(ml_dtypes.bfloat16)
    mask_full = np.ascontiguousarray(source_mask).reshape(1, N)
    in_maps = []
    for c in range(CORES):
        c0 = c * COLS
        in_maps.append({
            "ew_block": np.ascontiguousarray(ew[:, c0:c0 + COLS]),
            "mask_own": np.ascontiguousarray(source_mask[c0:c0 + COLS]).reshape(1, COLS),
            "mask_full": mask_full,
        })
    return in_maps


def run(adjacency, edge_weights, source_mask, trace=False, **spmd_kwargs):
    from concourse import bass_utils

    nc = _get_nc()
    in_maps = _make_in_maps(adjacency, edge_weights, source_mask)
    res = bass_utils.run_bass_kernel_spmd(
        nc, in_maps, core_ids=list(range(CORES)), trace=trace, **spmd_kwargs,
    )
    out = np.concatenate([res.results[c]["d_out"].reshape(COLS) for c in range(CORES)])
    return out.astype(np.float32), res


def kernel(adjacency, edge_weights, source_mask):
    out, _ = run(adjacency, edge_weights, source_mask, trace=False)
    return out


def build_baseline():
    """Trivial copy NEFF with the same I/O count — measures dispatch overhead."""
    import concourse.bacc as bacc
    import concourse.mybir as mybir
    import concourse.tile as tile

    f32 = mybir.dt.float32

    nc = bacc.Bacc(
        "TRN2",
        target_bir_lowering=False,
        debug=False,
        enable_asserts=False,
        num_devices=CORES,
    )
    x = nc.dram_tensor("x", [1, COLS], f32, kind="ExternalInput")
    y = nc.dram_tensor("y", [1, COLS], f32, kind="ExternalOutput")
    with tile.TileContext(nc) as tc:
        with tc.tile_pool(name="p", bufs=1) as pool:
            t = pool.tile([1, COLS], f32)
            nc.sync.dma_start(t[0:1, :], x[0:1, :])
            nc.sync.dma_start(y[0:1, :], t[0:1, :])
    nc.compile()
    in_maps = [{"x": np.zeros((1, COLS), np.float32)} for _ in range(CORES)]
    return nc, in_maps


# revision 12
# speedup vs baseline: 1.1034x; 1.1034x over previous
"""Trainium2 Bass kernel for nn_DifferentiablePathfinder.

Reference computation (N=8192, 20 iterations, tau=0.1):
    d0 = where(mask>0, 0, 100)
    effw = where(adj>0, W, 100)
    repeat 20x: d = min(d, -tau * logsumexp(-(d[:,None] + effw)/tau, axis=0))

Reformulation in linear ("q") space: with E = exp(-effw/tau) (zero where no
edge) and q = exp(-d/tau), one iteration is exactly

    q <- max(q, E^T q)        (elementwise max == min in d-space)

i.e. a repeated matvec with a FIXED matrix.  d never converges here (softmin
over ~4k candidates drags every distance down ~0.6/iter), so q would overflow
f32.  We rescale q every iteration (alternating 2^-9 / 2^-8, exact in fp,
which also keeps q in fp8's normal range [~0.7, ~7]) and track the
accumulated offset as a compile-time constant:

    stored q_t = exp(-(d_t - m_t)/tau),  m_{t+1} = m_t + tau*ln(scale_t)
    q_{t+1} = max(q_t, E^T q_t) * scale_t
    final d = m_T - tau * ln(q_T)

Sharding: E is column-sharded across 8 cores (1024 cols each).  The host
pre-merges adjacency+weights into ew = where(adj>0, W, 100) in bf16 (pure
input prep; 16 MB/core instead of 64 MB of f32 W + int32 adj, cutting the
initial HBM load 4x).  Each core keeps its [8192, 1024] block of
E = exp(-ew/tau) resident in SBUF as fp8-e4m3 (8 MB, built by the scalar
engine's Exp directly into the fp8 DoubleRow plane layout), and computes
s = E^T q on the tensor engine in DoubleRow mode (32 K-chunks of 256 rows
accumulated in PSUM f32).

Wave-pipelined AllGather: the per-iteration AG roundtrip (~8-10 us: DVE
scale+cast, DMA to the DRAM bounce buffer, TOPSP trigger, ~5 us collective,
DMA back, max) is hidden behind the ~17 us of matmuls by splitting each
iteration into two column waves:

  - q[p*64+k] lives at SBUF partition p, col k, split as q8a (k<32) and
    q8b (k>=32).  DoubleRow chunk c2 pairs cols (c2, c2+16) for c2<16 and
    (c2+16, c2+32) for c2>=16 (16 B apart - the minimum aligned interleave
    step), so chunks 0..15 read only q8a and 16..31 only q8b.
  - output columns are stored/computed u-ordered: group A = {j: j%64<32}
    (these feed every core's q8a), group B = the rest.  After AllGather of
    a group, each SBUF row's 32 bytes are contiguous in the gather buffer
    (flat addr = 32p + k), so the receive is a single clean DMA.
  - schedule per iteration: psA chunks 0..15 | psB 0..7 | psA 16..31 ->
    AG_A fires at ~60% of the burst | psB 8..31 -> AG_B at the end.  The
    next iteration needs AG_A at its start and AG_B only ~6 us in.
  - the elementwise max runs on the RECEIVE side ([128,32] tiles, 128-way
    parallel) against the pre-scaled previous q; receive DMA + max sit on
    the same (gpsimd) queue to avoid a cross-engine semaphore hop.

The first collective also absorbs the one-time ~55-80 us cross-core
dispatch-skew barrier; the E build + iteration 0 (~60 us) overlap most of
it, and no other work depends on a collective before that point.

Accuracy vs f32 reference: ~3e-4 relative (fp8 E quantization dominates;
errors average over ~2k terms per dot product; bf16 ew adds ~0.2% weight
rounding, far below fp8's 3%).

NOTE: all DRAM tensors and every AP passed to DMA are kept strictly 2-D+ -
1-D APs (e.g. `t[0, :]`) produce NEFFs that fail to load / wedge the device
on this environment.  tensor_tensor_reduce also fails at runtime here; use
separate max + scale ops.
"""

import numpy as np

# ---------------------------------------------------------------- constants
N = 8192
CORES = 8
COLS = N // CORES          # 1024 columns per core
P = 128                    # partitions
KPP = N // P               # 64 q entries per partition
CH2 = KPP // 2             # 32 DoubleRow chunks of 256 rows
HALF = COLS // 2           # 512 (output-group size / PSUM bank)
T = 20                     # iterations (fixed; reference never converges)
TAU = 0.1
INF_W = 100.0              # no-edge marker in ew (exp(-1000) == 0 in fp8)
SCALES = [1.0 / 512.0 if t % 2 == 0 else 1.0 / 256.0 for t in range(T)]
M_T = TAU * float(np.sum(np.log(SCALES)))   # log-offset after T iters

RPS = 4                    # rows per slab (per partition)
NSLAB = KPP // RPS         # 16 slabs


def _slab_dest(s):
    """E3 (chunk-range start, plane) written by slab s (rows k = 4s..4s+3)."""
    k0 = s * RPS
    if k0 < 16:
        return k0, 0            # c2 = k, plane 0
    if k0 < 32:
        return k0 - 16, 1       # c2 = k-16, plane 1
    if k0 < 48:
        return k0 - 16, 0       # c2 = k-16, plane 0
    return k0 - 32, 1           # c2 = k-32, plane 1


_CACHE = {}


def _build():
    """Build + compile the SPMD Bass program (same program on all 8 cores)."""
    import concourse.bacc as bacc
    import concourse.mybir as mybir
    import concourse.tile as tile

    f32 = mybir.dt.float32
    bf16 = mybir.dt.bfloat16
    fp8 = mybir.dt.float8e4
    i32 = mybir.dt.int32
    DR = mybir.MatmulPerfMode.DoubleRow

    nc = bacc.Bacc(
        "TRN2",
        target_bir_lowering=False,
        debug=False,
        enable_asserts=False,
        num_devices=CORES,
    )

    ew_dram = nc.dram_tensor("ew_block", [N, COLS], bf16, kind="ExternalInput")
    maskown_dram = nc.dram_tensor("mask_own", [1, COLS], i32, kind="ExternalInput")
    maskfull_dram = nc.dram_tensor("mask_full", [1, N], i32, kind="ExternalInput")
    d_dram = nc.dram_tensor("d_out", [1, COLS], f32, kind="ExternalOutput")

    # slab view: slab s holds rows {p*64 + 4s + r : r in 0..3} on partition p -
    # 4 consecutive rows per partition = one contiguous 8 KB DRAM run per
    # partition (bigger runs lift the DMA-engine rate substantially)
    ew_r = ew_dram.rearrange("(p s r) c -> s p (r c)", s=NSLAB, r=RPS)

    with tile.TileContext(nc) as tc:
        with (
            tc.tile_pool(name="resident", bufs=1) as rpool,
            tc.tile_pool(name="stage", bufs=3) as spool,
            tc.tile_pool(name="qpool", bufs=2) as qpool,
            tc.tile_pool(name="psum", bufs=2, space="PSUM") as ppool,
            tc.tile_pool(name="dram", bufs=2, space="DRAM") as dpool,
        ):
            # resident E block, fp8 DoubleRow planes: 64 KB/partition.
            # columns u-ordered: u<512 <-> output group A (j = 64*(u//32)+u%32)
            E3 = rpool.tile([P, CH2, 2, COLS], fp8)

            # ---------------- initial q from source mask (no collective) --
            maskown_sb = spool.tile([1, COLS], i32, tag="mskown", bufs=1)
            nc.sync.dma_start(maskown_sb[0:1, :], maskown_dram[0:1, :])
            qp = qpool.tile([1, COLS], f32, tag="qp")
            nc.vector.tensor_copy(qp[0:1, :], maskown_sb[0:1, :])  # int32 -> f32

            mskfull_sb = spool.tile([P, KPP], i32, tag="mskfull", bufs=1)
            nc.sync.dma_start(
                mskfull_sb[:, :],
                maskfull_dram.rearrange("a (p k) -> (a p) k", k=KPP),
            )
            q8a = qpool.tile([P, CH2], fp8, tag="q8a")
            q8b = qpool.tile([P, CH2], fp8, tag="q8b")
            nc.vector.tensor_copy(q8a[:, :], mskfull_sb[:, 0:CH2])   # i32 -> fp8
            nc.vector.tensor_copy(q8b[:, :], mskfull_sb[:, CH2:KPP])

            # ---------------- build resident E = exp(-ew/tau) -------------
            # slab order pairs plane-0/plane-1 sources so DoubleRow chunks
            # become ready in schedule order; iteration 0 overlaps the build
            slab_order = []
            for s in range(4):
                slab_order += [s, s + 4]
            for s in range(8, 12):
                slab_order += [s, s + 4]
            slab_tiles = {}
            for i, s in enumerate(slab_order):
                ewst = spool.tile([P, RPS * COLS], bf16, tag="ewst", bufs=7)
                # alternate HW-DGE (sync) and SW-DGE (gpsimd) queues
                if i % 2 == 0:
                    nc.sync.dma_start(ewst[:, :], ew_r[s])
                else:
                    nc.gpsimd.dma_start(ewst[:, :], ew_r[s])
                slab_tiles[i] = (s, ewst)

            def emit_act(i, g):
                s, ewst = slab_tiles[i]
                c0, pl = _slab_dest(s)
                ewst4 = ewst.rearrange("p (r b j) -> p r b j", r=RPS, j=KPP)
                nc.scalar.activation(
                    E3[:, c0:c0 + RPS, pl, g * HALF:(g + 1) * HALF]
                    .rearrange("p c (b j) -> p c b j", j=CH2),
                    ewst4[:, :, :, g * CH2:(g + 1) * CH2],
                    mybir.ActivationFunctionType.Exp,
                    bias=0.0, scale=-1.0 / TAU,
                )

            # A-wave (output group 0) exps run ~5 slabs ahead of B-wave, so
            # iteration 0's psA - and with it the first real AllGather -
            # completes ~10 us earlier.  7 staging bufs bound the lookahead.
            LOOK = 5
            for i in range(NSLAB):
                emit_act(i, 0)
                if i >= LOOK:
                    emit_act(i - LOOK, 1)
            for i in range(NSLAB - LOOK, NSLAB):
                emit_act(i, 1)

            # ---------------- 20 iterations ------------------------------
            # chunk c2 -> q8a cols (c2, c2+16) for c2<16; q8b (c2-16, c2)
            def lhsT_of(c2):
                if c2 < 16:
                    return q8a[:, c2:c2 + 17:16].rearrange(
                        "p (a m) -> p a m", a=2)
                b0 = c2 - 16
                return q8b[:, b0:b0 + 17:16].rearrange("p (a m) -> p a m", a=2)

            def mm_group(ps, grp, c2s):
                for c2 in c2s:
                    nc.tensor.matmul(
                        ps[0:1, :], lhsT_of(c2),
                        E3[:, c2, :, grp * HALF:(grp + 1) * HALF],
                        start=(c2 == 0), stop=(c2 == CH2 - 1),
                        perf_mode=DR,
                    )

            for t in range(T):
                ps_a = ppool.tile([1, HALF], f32, tag="psa")
                ps_b = ppool.tile([1, HALF], f32, tag="psb")
                last = t == T - 1

                # pre-scaled previous q for the receive-side max; DVE runs
                # these while the matmuls stream
                if not last:
                    q8sa = qpool.tile([P, CH2], fp8, tag="q8sa")
                    q8sb = qpool.tile([P, CH2], fp8, tag="q8sb")
                    nc.vector.tensor_scalar_mul(q8sa[:, :], q8a[:, :], SCALES[t])
                    nc.vector.tensor_scalar_mul(q8sb[:, :], q8b[:, :], SCALES[t])

                # ---- matmul schedule: A-wave output first, B-chunks late
                mm_group(ps_a, 0, range(0, 16))
                mm_group(ps_b, 1, range(0, 8))
                mm_group(ps_a, 0, range(16, 32))
                # tail A: one scale+cast, DMA out, trigger (sync queue)
                if not last:
                    # scale+cast split across DVE and the idle scalar engine:
                    # halves run in parallel, halving this critical hop
                    q8cca = qpool.tile([1, HALF], fp8, tag="q8cca")
                    nc.vector.tensor_scalar_mul(
                        q8cca[0:1, 0:HALF // 2], ps_a[0:1, 0:HALF // 2], SCALES[t])
                    nc.scalar.activation(
                        q8cca[0:1, HALF // 2:HALF], ps_a[0:1, HALF // 2:HALF],
                        mybir.ActivationFunctionType.Copy,
                        bias=0.0, scale=SCALES[t],
                    )
                    cc_ina = dpool.tile([1, HALF], fp8, tag="ccina")
                    nc.sync.dma_start(cc_ina[0:1, :], q8cca[0:1, :])
                    cc_outa = dpool.tile([CORES, HALF], fp8, tag="ccouta",
                                         addr_space="Shared")
                    nc.gpsimd.collective_compute(
                        "AllGather", mybir.AluOpType.bypass,
                        replica_groups=[list(range(CORES))],
                        ins=[cc_ina[0:1, :].opt()],
                        outs=[cc_outa[:, :].opt()],
                    )
                mm_group(ps_b, 1, range(8, 32))
                if not last:
                    q8ccb = qpool.tile([1, HALF], fp8, tag="q8ccb")
                    nc.vector.tensor_scalar_mul(
                        q8ccb[0:1, 0:HALF // 2], ps_b[0:1, 0:HALF // 2], SCALES[t])
                    nc.scalar.activation(
                        q8ccb[0:1, HALF // 2:HALF], ps_b[0:1, HALF // 2:HALF],
                        mybir.ActivationFunctionType.Copy,
                        bias=0.0, scale=SCALES[t],
                    )
                    cc_inb = dpool.tile([1, HALF], fp8, tag="ccinb")
                    nc.sync.dma_start(cc_inb[0:1, :], q8ccb[0:1, :])
                    cc_outb = dpool.tile([CORES, HALF], fp8, tag="ccoutb",
                                         addr_space="Shared")
                    nc.gpsimd.collective_compute(
                        "AllGather", mybir.AluOpType.bypass,
                        replica_groups=[list(range(CORES))],
                        ins=[cc_inb[0:1, :].opt()],
                        outs=[cc_outb[:, :].opt()],
                    )
                    # receive: DMA on the (idle) scalar queue so neither
                    # the gpsimd trigger queue nor the DVE stalls; wave A
                    # first - it gates the next burst
                    agta = qpool.tile([P, CH2], fp8, tag="agta")
                    nc.scalar.dma_start(
                        agta[:, :],
                        cc_outa.rearrange("c (pp k) -> (c pp) k", k=CH2),
                    )
                    q8a_new = qpool.tile([P, CH2], fp8, tag="q8a")
                    nc.vector.tensor_tensor(
                        q8a_new[:, :], agta[:, :], q8sa[:, :],
                        mybir.AluOpType.max)
                    agtb = qpool.tile([P, CH2], fp8, tag="agtb")
                    nc.scalar.dma_start(
                        agtb[:, :],
                        cc_outb.rearrange("c (pp k) -> (c pp) k", k=CH2),
                    )
                    q8b_new = qpool.tile([P, CH2], fp8, tag="q8b")
                    nc.vector.tensor_tensor(
                        q8b_new[:, :], agtb[:, :], q8sb[:, :],
                        mybir.AluOpType.max)

                # ---- f32 master copy of own slice (j-ordered; feeds only
                # the final output, fully off the AG critical path)
                qp_s = qpool.tile([1, COLS], f32, tag="qps")
                nc.vector.tensor_scalar_mul(qp_s[0:1, :], qp[0:1, :], SCALES[t])
                qp_new = qpool.tile([1, COLS], f32, tag="qp")
                for grp, ps in ((0, ps_a), (1, ps_b)):
                    sps = qpool.tile([1, HALF], f32, tag="sps")
                    nc.vector.tensor_scalar_mul(sps[0:1, :], ps[0:1, :], SCALES[t])
                    jview = lambda ap: ap.rearrange(
                        "a (b j) -> a b j", j=KPP)[:, :, grp * CH2:(grp + 1) * CH2]
                    nc.vector.tensor_tensor(
                        jview(qp_new[0:1, :]), jview(qp_s[0:1, :]),
                        sps[0:1, :].rearrange("a (b j) -> a b j", j=CH2),
                        mybir.AluOpType.max,
                    )
                qp = qp_new
                if not last:
                    q8a = q8a_new
                    q8b = q8b_new

            # ---------------- final: d = m_T - tau*ln(q), clamp to 100 ----
            lnq = qpool.tile([1, COLS], f32, tag="lnq", bufs=1)
            nc.scalar.activation(
                lnq[0:1, :], qp[0:1, :], mybir.ActivationFunctionType.Ln,
            )
            dfin = qpool.tile([1, COLS], f32, tag="dfin", bufs=1)
            nc.scalar.activation(
                dfin[0:1, :], lnq[0:1, :], mybir.ActivationFunctionType.Copy,
                bias=M_T, scale=-TAU,
            )
            dcl = qpool.tile([1, COLS], f32, tag="dcl", bufs=1)
            nc.vector.tensor_scalar_min(dcl[0:1, :], dfin[0:1, :], 100.0)
            nc.sync.dma_start(d_dram[0:1, :], dcl[0:1, :])

    nc.compile()
    return nc


def _get_nc():
    if "nc" not in _CACHE:
        _CACHE["nc"] = _build()
    return _CACHE["nc"]


def _make_in_maps(adjacency, edge_weights, source_mask):
    import ml_dtypes

    adjacency = np.asarray(adjacency, dtype=np.int32)
    edge_weights = np.asarray(edge_weights, dtype=np.float32)
    source_mask = np.asarray(source_mask, dtype=np.int32)
    # input prep (pure sharding/packing): effective weights in bf16
    ew = np.where(adjacency > 0, edge_weights, np.float32(INF_W))
    ew = ew.astype(ml_dtypes.bfloat16)
    mask_full = np.ascontiguousarray(source_mask).reshape(1, N)
    in_maps = []
    for c in range(CORES):
        c0 = c * COLS
        in_maps.append({
            "ew_block": np.ascontiguousarray(ew[:, c0:c0 + COLS]),
            "mask_own": np.ascontiguousarray(source_mask[c0:c0 + COLS]).reshape(1, COLS),
            "mask_full": mask_full,
        })
    return in_maps


def run(adjacency, edge_weights, source_mask, trace=False, **spmd_kwargs):
    from concourse import bass_utils

    nc = _get_nc()
    in_maps = _make_in_maps(adjacency, edge_weights, source_mask)
    res = bass_utils.run_bass_kernel_spmd(
        nc, in_maps, core_ids=list(range(CORES)), trace=trace, **spmd_kwargs,
    )
    out = np.concatenate([res.results[c]["d_out"].reshape(COLS) for c in range(CORES)])
    return out.astype(np.float32), res


def kernel(adjacency, edge_weights, source_mask):
    out, _ = run(adjacency, edge_weights, source_mask, trace=False)
    return out


def build_baseline():
    """Trivial copy NEFF with the same I/O count — measures dispatch overhead."""
    import concourse.bacc as bacc
    import concourse.mybir as mybir
    import concourse.tile as tile

    f32 = mybir.dt.float32

    nc = bacc.Bacc(
        "TRN2",
        target_bir_lowering=False,
        debug=False,
        enable_asserts=False,
        num_devices=CORES,
    )
    x = nc.dram_tensor("x", [1, COLS], f32, kind="ExternalInput")
    y = nc.dram_tensor("y", [1, COLS], f32, kind="ExternalOutput")
    with tile.TileContext(nc) as tc:
        with tc.tile_pool(name="p", bufs=1) as pool:
            t = pool.tile([1, COLS], f32)
            nc.sync.dma_start(t[0:1, :], x[0:1, :])
            nc.sync.dma_start(y[0:1, :], t[0:1, :])
    nc.compile()
    in_maps = [{"x": np.zeros((1, COLS), np.float32)} for _ in range(CORES)]
    return nc, in_maps


# revision 13
# speedup vs baseline: 1.1530x; 1.0450x over previous
"""Trainium2 Bass kernel for nn_DifferentiablePathfinder.

Reference computation (N=8192, 20 iterations, tau=0.1):
    d0 = where(mask>0, 0, 100)
    effw = where(adj>0, W, 100)
    repeat 20x: d = min(d, -tau * logsumexp(-(d[:,None] + effw)/tau, axis=0))

Reformulation in linear ("q") space: with E = exp(-effw/tau) (zero where no
edge) and q = exp(-d/tau), one iteration is exactly

    q <- max(q, E^T q)        (elementwise max == min in d-space)

i.e. a repeated matvec with a FIXED matrix.  d never converges here (softmin
over ~4k candidates drags every distance down ~0.6/iter), so q would overflow
f32.  We rescale q every iteration (alternating 2^-9 / 2^-8, exact in fp,
which also keeps q in fp8's normal range [~0.7, ~7]) and track the
accumulated offset as a compile-time constant:

    stored q_t = exp(-(d_t - m_t)/tau),  m_{t+1} = m_t + tau*ln(scale_t)
    q_{t+1} = max(q_t, E^T q_t) * scale_t
    final d = m_T - tau * ln(q_T)

Sharding: E is column-sharded across 8 cores (1024 cols each).  The host
pre-merges adjacency+weights into ew = where(adj>0, W, 100) in bf16 (pure
input prep; 16 MB/core instead of 64 MB of f32 W + int32 adj, cutting the
initial HBM load 4x).  Each core keeps its [8192, 1024] block of
E = exp(-ew/tau) resident in SBUF as fp8-e4m3 (8 MB, built by the scalar
engine's Exp directly into the fp8 DoubleRow plane layout), and computes
s = E^T q on the tensor engine in DoubleRow mode (32 K-chunks of 256 rows
accumulated in PSUM f32).

Wave-pipelined AllGather: the per-iteration AG roundtrip (~8-10 us: DVE
scale+cast, DMA to the DRAM bounce buffer, TOPSP trigger, ~5 us collective,
DMA back, max) is hidden behind the ~17 us of matmuls by splitting each
iteration into two column waves:

  - q[p*64+k] lives at SBUF partition p, col k, split as q8a (k<32) and
    q8b (k>=32).  DoubleRow chunk c2 pairs cols (c2, c2+16) for c2<16 and
    (c2+16, c2+32) for c2>=16 (16 B apart - the minimum aligned interleave
    step), so chunks 0..15 read only q8a and 16..31 only q8b.
  - output columns are stored/computed u-ordered: group A = {j: j%64<32}
    (these feed every core's q8a), group B = the rest.  After AllGather of
    a group, each SBUF row's 32 bytes are contiguous in the gather buffer
    (flat addr = 32p + k), so the receive is a single clean DMA.
  - schedule per iteration: psA chunks 0..15 | psB 0..7 | psA 16..31 ->
    AG_A fires at ~60% of the burst | psB 8..31 -> AG_B at the end.  The
    next iteration needs AG_A at its start and AG_B only ~6 us in.
  - the elementwise max runs on the RECEIVE side ([128,32] tiles, 128-way
    parallel) against the pre-scaled previous q; receive DMA + max sit on
    the same (gpsimd) queue to avoid a cross-engine semaphore hop.

The first collective also absorbs the one-time ~55-80 us cross-core
dispatch-skew barrier; the E build + iteration 0 (~60 us) overlap most of
it, and no other work depends on a collective before that point.

Accuracy vs f32 reference: ~3e-4 relative (fp8 E quantization dominates;
errors average over ~2k terms per dot product; bf16 ew adds ~0.2% weight
rounding, far below fp8's 3%).

NOTE: all DRAM tensors and every AP passed to DMA are kept strictly 2-D+ -
1-D APs (e.g. `t[0, :]`) produce NEFFs that fail to load / wedge the device
on this environment.  tensor_tensor_reduce also fails at runtime here; use
separate max + scale ops.
"""

import numpy as np

# ---------------------------------------------------------------- constants
N = 8192
CORES = 8
COLS = N // CORES          # 1024 columns per core
P = 128                    # partitions
KPP = N // P               # 64 q entries per partition
CH2 = KPP // 2             # 32 DoubleRow chunks of 256 rows
HALF = COLS // 2           # 512 (output-group size / PSUM bank)
T = 20                     # iterations (fixed; reference never converges)
TAU = 0.1
INF_W = 100.0              # no-edge marker in ew (exp(-1000) == 0 in fp8)
SCALES = [1.0 / 512.0 if t % 2 == 0 else 1.0 / 256.0 for t in range(T)]
M_T = TAU * float(np.sum(np.log(SCALES)))   # log-offset after T iters

RPS = 4                    # rows per slab (per partition)
NSLAB = KPP // RPS         # 16 slabs


def _slab_dest(s):
    """E3 (chunk-range start, plane) written by slab s (rows k = 4s..4s+3)."""
    k0 = s * RPS
    if k0 < 16:
        return k0, 0            # c2 = k, plane 0
    if k0 < 32:
        return k0 - 16, 1       # c2 = k-16, plane 1
    if k0 < 48:
        return k0 - 16, 0       # c2 = k-16, plane 0
    return k0 - 32, 1           # c2 = k-32, plane 1


_CACHE = {}


def _build():
    """Build + compile the SPMD Bass program (same program on all 8 cores)."""
    import concourse.bacc as bacc
    import concourse.mybir as mybir
    import concourse.tile as tile

    f32 = mybir.dt.float32
    bf16 = mybir.dt.bfloat16
    fp8 = mybir.dt.float8e4
    i32 = mybir.dt.int32
    DR = mybir.MatmulPerfMode.DoubleRow

    nc = bacc.Bacc(
        "TRN2",
        target_bir_lowering=False,
        debug=False,
        enable_asserts=False,
        num_devices=CORES,
    )

    ew_dram = nc.dram_tensor("ew_block", [N, COLS], bf16, kind="ExternalInput")
    maskown_dram = nc.dram_tensor("mask_own", [1, COLS], i32, kind="ExternalInput")
    maskfull_dram = nc.dram_tensor("mask_full", [1, N], i32, kind="ExternalInput")
    d_dram = nc.dram_tensor("d_out", [1, COLS], f32, kind="ExternalOutput")

    # slab view: slab s holds rows {p*64 + 4s + r : r in 0..3} on partition p -
    # 4 consecutive rows per partition = one contiguous 8 KB DRAM run per
    # partition (bigger runs lift the DMA-engine rate substantially)
    ew_r = ew_dram.rearrange("(p s r) c -> s p (r c)", s=NSLAB, r=RPS)

    with tile.TileContext(nc) as tc:
        with (
            tc.tile_pool(name="resident", bufs=1) as rpool,
            tc.tile_pool(name="stage", bufs=3) as spool,
            tc.tile_pool(name="qpool", bufs=2) as qpool,
            tc.tile_pool(name="psum", bufs=2, space="PSUM") as ppool,
            tc.tile_pool(name="dram", bufs=2, space="DRAM") as dpool,
        ):
            # resident E block, fp8 DoubleRow planes: 64 KB/partition.
            # columns u-ordered: u<512 <-> output group A (j = 64*(u//32)+u%32)
            E3 = rpool.tile([P, CH2, 2, COLS], fp8)

            # ---------------- initial q from source mask (no collective) --
            maskown_sb = spool.tile([1, COLS], i32, tag="mskown", bufs=1)
            nc.sync.dma_start(maskown_sb[0:1, :], maskown_dram[0:1, :])
            qp = qpool.tile([1, COLS], f32, tag="qp")
            nc.vector.tensor_copy(qp[0:1, :], maskown_sb[0:1, :])  # int32 -> f32

            mskfull_sb = spool.tile([P, KPP], i32, tag="mskfull", bufs=1)
            nc.sync.dma_start(
                mskfull_sb[:, :],
                maskfull_dram.rearrange("a (p k) -> (a p) k", k=KPP),
            )
            q8a = qpool.tile([P, CH2], fp8, tag="q8a")
            q8b = qpool.tile([P, CH2], fp8, tag="q8b")
            nc.vector.tensor_copy(q8a[:, :], mskfull_sb[:, 0:CH2])   # i32 -> fp8
            nc.vector.tensor_copy(q8b[:, :], mskfull_sb[:, CH2:KPP])

            # ---------------- build resident E = exp(-ew/tau) -------------
            # slab order pairs plane-0/plane-1 sources so DoubleRow chunks
            # become ready in schedule order; iteration 0 overlaps the build
            slab_order = []
            for s in range(4):
                slab_order += [s, s + 4]
            for s in range(8, 12):
                slab_order += [s, s + 4]
            slab_tiles = {}
            for i, s in enumerate(slab_order):
                ewst = spool.tile([P, RPS * COLS], bf16, tag="ewst", bufs=7)
                # alternate HW-DGE (sync) and SW-DGE (gpsimd) queues
                if i % 2 == 0:
                    nc.sync.dma_start(ewst[:, :], ew_r[s])
                else:
                    nc.gpsimd.dma_start(ewst[:, :], ew_r[s])
                slab_tiles[i] = (s, ewst)

            def emit_act(i, g):
                s, ewst = slab_tiles[i]
                c0, pl = _slab_dest(s)
                ewst4 = ewst.rearrange("p (r b j) -> p r b j", r=RPS, j=KPP)
                nc.scalar.activation(
                    E3[:, c0:c0 + RPS, pl, g * HALF:(g + 1) * HALF]
                    .rearrange("p c (b j) -> p c b j", j=CH2),
                    ewst4[:, :, :, g * CH2:(g + 1) * CH2],
                    mybir.ActivationFunctionType.Exp,
                    bias=0.0, scale=-1.0 / TAU,
                )

            # A-wave (output group 0) exps run ~5 slabs ahead of B-wave, so
            # iteration 0's psA - and with it the first real AllGather -
            # completes ~10 us earlier.  7 staging bufs bound the lookahead.
            LOOK = 5
            for i in range(NSLAB):
                emit_act(i, 0)
                if i >= LOOK:
                    emit_act(i - LOOK, 1)
            for i in range(NSLAB - LOOK, NSLAB):
                emit_act(i, 1)

            # ---------------- 20 iterations ------------------------------
            # chunk c2 -> q8a cols (c2, c2+16) for c2<16; q8b (c2-16, c2)
            def lhsT_of(c2):
                if c2 < 16:
                    return q8a[:, c2:c2 + 17:16].rearrange(
                        "p (a m) -> p a m", a=2)
                b0 = c2 - 16
                return q8b[:, b0:b0 + 17:16].rearrange("p (a m) -> p a m", a=2)

            def mm_group(ps, grp, c2s):
                for c2 in c2s:
                    nc.tensor.matmul(
                        ps[0:1, :], lhsT_of(c2),
                        E3[:, c2, :, grp * HALF:(grp + 1) * HALF],
                        start=(c2 == 0), stop=(c2 == CH2 - 1),
                        perf_mode=DR,
                    )

            for t in range(T):
                ps_a = ppool.tile([1, HALF], f32, tag="psa")
                ps_b = ppool.tile([1, HALF], f32, tag="psb")
                last = t == T - 1

                # pre-scaled previous q for the receive-side max; DVE runs
                # these while the matmuls stream
                if not last:
                    q8sa = qpool.tile([P, CH2], fp8, tag="q8sa")
                    q8sb = qpool.tile([P, CH2], fp8, tag="q8sb")
                    nc.vector.tensor_scalar_mul(q8sa[:, :], q8a[:, :], SCALES[t])
                    nc.vector.tensor_scalar_mul(q8sb[:, :], q8b[:, :], SCALES[t])

                # ---- matmul schedule: A-wave output first, B-chunks late
                mm_group(ps_a, 0, range(0, 16))
                mm_group(ps_b, 1, range(0, 8))
                mm_group(ps_a, 0, range(16, 32))
                # tail A: one scale+cast, DMA out, trigger (sync queue)
                if not last:
                    q8cca = qpool.tile([1, HALF], fp8, tag="q8cca")
                    nc.vector.tensor_scalar_mul(q8cca[0:1, :], ps_a[0:1, :], SCALES[t])
                    cc_ina = dpool.tile([1, HALF], fp8, tag="ccina")
                    nc.sync.dma_start(cc_ina[0:1, :], q8cca[0:1, :])
                    cc_outa = dpool.tile([CORES, HALF], fp8, tag="ccouta",
                                         addr_space="Shared")
                    nc.gpsimd.collective_compute(
                        "AllGather", mybir.AluOpType.bypass,
                        replica_groups=[list(range(CORES))],
                        ins=[cc_ina[0:1, :].opt()],
                        outs=[cc_outa[:, :].opt()],
                    )
                mm_group(ps_b, 1, range(8, 32))
                if not last:
                    q8ccb = qpool.tile([1, HALF], fp8, tag="q8ccb")
                    nc.vector.tensor_scalar_mul(q8ccb[0:1, :], ps_b[0:1, :], SCALES[t])
                    cc_inb = dpool.tile([1, HALF], fp8, tag="ccinb")
                    nc.sync.dma_start(cc_inb[0:1, :], q8ccb[0:1, :])
                    cc_outb = dpool.tile([CORES, HALF], fp8, tag="ccoutb",
                                         addr_space="Shared")
                    nc.gpsimd.collective_compute(
                        "AllGather", mybir.AluOpType.bypass,
                        replica_groups=[list(range(CORES))],
                        ins=[cc_inb[0:1, :].opt()],
                        outs=[cc_outb[:, :].opt()],
                    )
                    # receive: DMA on the (idle) scalar queue so neither
                    # the gpsimd trigger queue nor the DVE stalls; wave A
                    # first - it gates the next burst
                    agta = qpool.tile([P, CH2], fp8, tag="agta")
                    nc.scalar.dma_start(
                        agta[:, :],
                        cc_outa.rearrange("c (pp k) -> (c pp) k", k=CH2),
                    )
                    q8a_new = qpool.tile([P, CH2], fp8, tag="q8a")
                    nc.vector.tensor_tensor(
                        q8a_new[:, :], agta[:, :], q8sa[:, :],
                        mybir.AluOpType.max)
                    agtb = qpool.tile([P, CH2], fp8, tag="agtb")
                    nc.scalar.dma_start(
                        agtb[:, :],
                        cc_outb.rearrange("c (pp k) -> (c pp) k", k=CH2),
                    )
                    q8b_new = qpool.tile([P, CH2], fp8, tag="q8b")
                    nc.vector.tensor_tensor(
                        q8b_new[:, :], agtb[:, :], q8sb[:, :],
                        mybir.AluOpType.max)

                # ---- f32 master copy of own slice (j-ordered; feeds only
                # the final output, fully off the AG critical path)
                qp_s = qpool.tile([1, COLS], f32, tag="qps")
                nc.vector.tensor_scalar_mul(qp_s[0:1, :], qp[0:1, :], SCALES[t])
                qp_new = qpool.tile([1, COLS], f32, tag="qp")
                for grp, ps in ((0, ps_a), (1, ps_b)):
                    sps = qpool.tile([1, HALF], f32, tag="sps")
                    nc.vector.tensor_scalar_mul(sps[0:1, :], ps[0:1, :], SCALES[t])
                    jview = lambda ap: ap.rearrange(
                        "a (b j) -> a b j", j=KPP)[:, :, grp * CH2:(grp + 1) * CH2]
                    nc.vector.tensor_tensor(
                        jview(qp_new[0:1, :]), jview(qp_s[0:1, :]),
                        sps[0:1, :].rearrange("a (b j) -> a b j", j=CH2),
                        mybir.AluOpType.max,
                    )
                qp = qp_new
                if not last:
                    q8a = q8a_new
                    q8b = q8b_new

            # ---------------- final: d = m_T - tau*ln(q), clamp to 100 ----
            lnq = qpool.tile([1, COLS], f32, tag="lnq", bufs=1)
            nc.scalar.activation(
                lnq[0:1, :], qp[0:1, :], mybir.ActivationFunctionType.Ln,
            )
            dfin = qpool.tile([1, COLS], f32, tag="dfin", bufs=1)
            nc.scalar.activation(
                dfin[0:1, :], lnq[0:1, :], mybir.ActivationFunctionType.Copy,
                bias=M_T, scale=-TAU,
            )
            dcl = qpool.tile([1, COLS], f32, tag="dcl", bufs=1)
            nc.vector.tensor_scalar_min(dcl[0:1, :], dfin[0:1, :], 100.0)
            nc.sync.dma_start(d_dram[0:1, :], dcl[0:1, :])

    nc.compile()
    return nc


def _get_nc():
    if "nc" not in _CACHE:
        _CACHE["nc"] = _build()
    return _CACHE["nc"]


def _make_in_maps(adjacency, edge_weights, source_mask):
    import ml_dtypes

    adjacency = np.asarray(adjacency, dtype=np.int32)
    edge_weights = np.asarray(edge_weights, dtype=np.float32)
    source_mask = np.asarray(source_mask, dtype=np.int32)
    # input prep (pure sharding/packing): effective weights in bf16
    ew = np.where(adjacency > 0, edge_weights, np.float32(INF_W))
    ew = ew.astype(ml_dtypes.bfloat16)
    mask_full = np.ascontiguousarray(source_mask).reshape(1, N)
    in_maps = []
    for c in range(CORES):
        c0 = c * COLS
        in_maps.append({
            "ew_block": np.ascontiguousarray(ew[:, c0:c0 + COLS]),
            "mask_own": np.ascontiguousarray(source_mask[c0:c0 + COLS]).reshape(1, COLS),
            "mask_full": mask_full,
        })
    return in_maps


def run(adjacency, edge_weights, source_mask, trace=False, **spmd_kwargs):
    from concourse import bass_utils

    nc = _get_nc()
    in_maps = _make_in_maps(adjacency, edge_weights, source_mask)
    res = bass_utils.run_bass_kernel_spmd(
        nc, in_maps, core_ids=list(range(CORES)), trace=trace, **spmd_kwargs,
    )
    out = np.concatenate([res.results[c]["d_out"].reshape(COLS) for c in range(CORES)])
    return out.astype(np.float32), res


def kernel(adjacency, edge_weights, source_mask):
    out, _ = run(adjacency, edge_weights, source_mask, trace=False)
    return out


def build_baseline():
    """Trivial copy NEFF with the same I/O count — measures dispatch overhead."""
    import concourse.bacc as bacc
    import concourse.mybir as mybir
    import concourse.tile as tile

    f32 = mybir.dt.float32

    nc = bacc.Bacc(
        "TRN2",
        target_bir_lowering=False,
        debug=False,
        enable_asserts=False,
        num_devices=CORES,
    )
    x = nc.dram_tensor("x", [1, COLS], f32, kind="ExternalInput")
    y = nc.dram_tensor("y", [1, COLS], f32, kind="ExternalOutput")
    with tile.TileContext(nc) as tc:
        with tc.tile_pool(name="p", bufs=1) as pool:
            t = pool.tile([1, COLS], f32)
            nc.sync.dma_start(t[0:1, :], x[0:1, :])
            nc.sync.dma_start(y[0:1, :], t[0:1, :])
    nc.compile()
    in_maps = [{"x": np.zeros((1, COLS), np.float32)} for _ in range(CORES)]
    return nc, in_maps
